# revision 3
# baseline (speedup 1.0000x reference)
"""MoE ConditionalFeedForward (SwiGLU, top-2 of 8 experts) on 8 TRN2 NeuronCores.

Strategy: expert-parallel. Core e owns expert e's weights (w1/w2/w3 slices).
The host routes tokens: for each expert, gather the tokens assigned to it
(padded to CAP, a multiple of 128), each core computes
    y = (silu(x @ w1[e].T) * (x @ w3[e].T)) @ w2[e].T
densely for its gathered tokens, and the host scatters rows back into the
[T, A, D] output.

Per-core kernel layout (all host-pretransposed so every DMA is contiguous):
  xt  [128, 8*CAP]   xt[p, k*CAP+j] = x_g[j, k*128+p]      (tokens, transposed)
  w13 [22, 128, 2048] w13[it,p,k*128+c] = w1[e][it*128+c, k*128+p], w3 at +1024
  w2s [22, 128, 1024] w2s[it,p,d] = w2[e][d, it*128+p]
  y   [CAP, 1024]    f32 output rows per gathered token

Phase A (per i-tile it of 22): h1T/h3T [128(i), CAP] = sum_k w.T @ x tiles in
PSUM, then hT = silu(h1)*h3 into SBUF. Phase B: y[t,:] accumulated over the
22 i-tiles in PSUM (6 banks: 3 t-tiles x 2 n-chunks of 512).
"""

import numpy as np
from contextlib import ExitStack

import concourse.bass as bass
import concourse.bacc as bacc
import concourse.mybir as mybir
import concourse.tile as tile
from concourse.bass_utils import run_bass_kernel_spmd

E, I, D = 8, 2816, 1024
N_CORES = 8
NI, ND = I // 128, D // 128  # 22, 8

# storage dtype for weights/activations on-device: "float32" (matmuls run as
# float32r) or "bfloat16" (half the HBM traffic)
DT_NAME = "float32"

_PROG_CACHE: dict = {}


def _build_program(cap: int, dt_name: str):
    # float32 inputs run the PE in float32r mode (full-rate at N>=256);
    # declare the tensors as float32r end-to-end so the BIR verifier sees
    # consistent provenance (np dtype is still float32).
    DT = mybir.dt.float32r if dt_name == "float32" else getattr(mybir.dt, dt_name)
    f32 = mybir.dt.float32
    nt = cap // 128
    nc = bacc.Bacc("TRN2", target_bir_lowering=False, debug=False)
    xt = nc.dram_tensor("xt", [128, ND * cap], DT, kind="ExternalInput").ap()
    w13 = nc.dram_tensor("w13", [NI, 128, 2 * D], DT, kind="ExternalInput").ap()
    w2s = nc.dram_tensor("w2s", [NI, 128, D], DT, kind="ExternalInput").ap()
    y = nc.dram_tensor("y", [cap, D], f32, kind="ExternalOutput").ap()

    mm = lambda ap: ap

    with tile.TileContext(nc) as tc, ExitStack() as ctx:
        xp = ctx.enter_context(tc.tile_pool(name="x", bufs=1))
        w13p = ctx.enter_context(tc.tile_pool(name="w13", bufs=3))
        hp = ctx.enter_context(tc.tile_pool(name="h", bufs=NI))
        silp = ctx.enter_context(tc.tile_pool(name="sil", bufs=2))
        w2p = ctx.enter_context(tc.tile_pool(name="w2", bufs=6))
        yp = ctx.enter_context(tc.tile_pool(name="y", bufs=2))

        xsb = xp.tile([128, ND * cap], DT)
        nc.sync.dma_start(xsb[:], xt[:])

        hts = []
        with tc.tile_pool(name="hps", bufs=2, space="PSUM") as hps:
            for it in range(NI):
                wt = w13p.tile([128, 2 * D], DT)
                nc.sync.dma_start(wt[:], w13[it])
                h1 = hps.tile([128, cap], f32, tag="h1")
                h3 = hps.tile([128, cap], f32, tag="h3")
                for k in range(ND):
                    nc.tensor.matmul(
                        h1[:],
                        mm(wt[:, k * 128 : (k + 1) * 128]),
                        mm(xsb[:, k * cap : (k + 1) * cap]),
                        start=(k == 0),
                        stop=(k == ND - 1),
                    )
                for k in range(ND):
                    nc.tensor.matmul(
                        h3[:],
                        mm(wt[:, D + k * 128 : D + (k + 1) * 128]),
                        mm(xsb[:, k * cap : (k + 1) * cap]),
                        start=(k == 0),
                        stop=(k == ND - 1),
                    )
                sil = silp.tile([128, cap], f32)
                nc.scalar.activation(
                    sil[:], h1[:], mybir.ActivationFunctionType.Silu
                )
                ht = hp.tile([128, cap], DT)
                nc.vector.tensor_mul(ht[:], sil[:], h3[:])
                hts.append(ht)

        with tc.tile_pool(name="yps", bufs=1, space="PSUM") as yps:
            ypt = [
                [
                    yps.tile(
                        [128, 512], f32, tag=f"yps_{t}_{n}", name=f"yps_{t}_{n}"
                    )
                    for n in range(2)
                ]
                for t in range(nt)
            ]
            for it in range(NI):
                w2t = w2p.tile([128, D], DT)
                nc.sync.dma_start(w2t[:], w2s[it])
                for t in range(nt):
                    for n in range(2):
                        nc.tensor.matmul(
                            ypt[t][n][:],
                            mm(hts[it][:, t * 128 : (t + 1) * 128]),
                            mm(w2t[:, n * 512 : (n + 1) * 512]),
                            start=(it == 0),
                            stop=(it == NI - 1),
                        )
            for t in range(nt):
                ysb = yp.tile([128, D], f32)
                for n in range(2):
                    nc.vector.tensor_copy(ysb[:, n * 512 : (n + 1) * 512], ypt[t][n][:])
                nc.sync.dma_start(y[t * 128 : (t + 1) * 128, :], ysb[:])

    nc.compile()
    return nc


def _get_program(cap: int, dt_name: str):
    key = (cap, dt_name)
    if key not in _PROG_CACHE:
        _PROG_CACHE[key] = _build_program(cap, dt_name)
    return _PROG_CACHE[key]


def _np_dt(dt_name: str):
    if dt_name == "float32":
        return np.float32
    import ml_dtypes

    return ml_dtypes.bfloat16


def _prep_weights(w1, w3, w2, dt_name):
    """Per-expert pretransposed/tiled weight arrays (see module docstring)."""
    npdt = _np_dt(dt_name)
    w13_all, w2s_all = [], []
    for e in range(E):
        # [I, D] -> [it, c, k, p] -> [it, p, k, c] -> [it, 128, 1024]
        a1 = w1[e].reshape(NI, 128, ND, 128).transpose(0, 3, 2, 1).reshape(NI, 128, D)
        a3 = w3[e].reshape(NI, 128, ND, 128).transpose(0, 3, 2, 1).reshape(NI, 128, D)
        w13_all.append(
            np.ascontiguousarray(np.concatenate([a1, a3], axis=2)).astype(npdt)
        )
        # w2[e] [D, I] -> T [I, D] -> [22, 128, 1024]
        w2s_all.append(
            np.ascontiguousarray(w2[e].T).reshape(NI, 128, D).astype(npdt)
        )
    return w13_all, w2s_all


def kernel(x, w1, w2, w3, expert_indices, _trace=False):
    x = np.asarray(x, dtype=np.float32)
    w1 = np.asarray(w1, dtype=np.float32)
    w2 = np.asarray(w2, dtype=np.float32)
    w3 = np.asarray(w3, dtype=np.float32)
    idx = np.asarray(expert_indices).astype(np.int64)
    T, A = idx.shape
    npdt = _np_dt(DT_NAME)

    flat = idx.ravel()  # position p = t*A + a -> expert id
    order = np.argsort(flat, kind="stable")
    counts = np.bincount(flat, minlength=E)
    offs = np.zeros(E + 1, dtype=np.int64)
    np.cumsum(counts, out=offs[1:])

    w13_all, w2s_all = _prep_weights(w1, w3, w2, DT_NAME)

    out = np.empty((T * A, D), dtype=np.float32)
    remaining = counts.copy()
    done = np.zeros(E, dtype=np.int64)
    last_res = None
    while remaining.max() > 0:
        cap = min(512, max(128, int(-(-remaining.max() // 128)) * 128))
        nc = _get_program(cap, DT_NAME)
        in_maps = []
        core_pos = []  # per-core flat positions handled this round
        for e in range(E):
            n = int(min(remaining[e], cap))
            pos = order[offs[e] + done[e] : offs[e] + done[e] + n]
            core_pos.append(pos)
            xg = np.zeros((cap, D), dtype=np.float32)
            xg[:n] = x[pos // A]
            # [cap, D] -> T [D, cap] -> [k, 128, cap] -> [128, k, cap]
            xt_host = np.ascontiguousarray(
                xg.T.reshape(ND, 128, cap).transpose(1, 0, 2)
            ).reshape(128, ND * cap).astype(npdt)
            in_maps.append({"xt": xt_host, "w13": w13_all[e], "w2s": w2s_all[e]})
            remaining[e] -= n
            done[e] += n
        last_res = run_bass_kernel_spmd(
            nc, in_maps, core_ids=list(range(N_CORES)), trace=_trace
        )
        for e in range(E):
            pos = core_pos[e]
            if len(pos):
                out[pos] = last_res.results[e]["y"][: len(pos)]

    result = out.reshape(T, A, D)
    if _trace:
        return result, last_res
    return result


# revision 4
# speedup vs baseline: 1.2752x; 1.2752x over previous
"""MoE ConditionalFeedForward (SwiGLU, top-2 of 8 experts) on 8 TRN2 NeuronCores.

Strategy: expert-parallel. Core e owns expert e's weights (w1/w2/w3 slices).
The host routes tokens: for each expert, gather the tokens assigned to it
(padded to CAP), each core computes
    y = (silu(x @ w1[e].T) * (x @ w3[e].T)) @ w2[e].T
densely for its gathered tokens, and the host scatters rows back into the
[T, A, D] output.

Per-core kernel layout (all host-pretransposed so every DMA is contiguous):
  xt  [128, 8*CAP]    xt[p, k*CAP+j] = x_g[j, k*128+p]     (tokens, transposed)
  w13 [22, 128, 2048] w13[it,p,k*128+c] = w1[e][it*128+c, k*128+p], w3 at +1024
  w2s [22, 128, 1024] w2s[it,p,d] = w2[e][d, it*128+p]
  yt  [128, 8*CAP]    f32, yt[p, k*CAP+j] = y_g[j, k*128+p] (output, transposed)

Phase A (per i-tile it of 22): h1T/h3T [128(i), CAP] = sum_k wT @ x tiles in
PSUM, then hT = silu(h1)*h3 into SBUF. Phase B (transposed): yT[d-tile] [128,
CAP] accumulated over the 22 i-tiles in PSUM (8 banks, one per d-tile), with
the w2 128x128 tile stationary and hT moving.
"""

import numpy as np
from contextlib import ExitStack

import concourse.bass as bass
import concourse.bacc as bacc
import concourse.mybir as mybir
import concourse.tile as tile
from concourse.bass_utils import run_bass_kernel_spmd

E, I, D = 8, 2816, 1024
N_CORES = 8
NI, ND = I // 128, D // 128  # 22, 8

# storage dtype for weights/activations on-device: "bfloat16" (half HBM
# traffic, full PE rate) or "float32" (matmuls run as float32r, 2 cyc/row)
DT_NAME = "bfloat16"

_PROG_CACHE: dict = {}


def _build_program(cap: int, dt_name: str):
    DT = mybir.dt.float32r if dt_name == "float32" else getattr(mybir.dt, dt_name)
    f32 = mybir.dt.float32
    nc = bacc.Bacc("TRN2", target_bir_lowering=False, debug=False)
    xt = nc.dram_tensor("xt", [128, ND * cap], DT, kind="ExternalInput").ap()
    w13 = nc.dram_tensor("w13", [NI, 128, 2 * D], DT, kind="ExternalInput").ap()
    w2s = nc.dram_tensor("w2s", [NI, 128, D], DT, kind="ExternalInput").ap()
    yt = nc.dram_tensor("yt", [128, ND * cap], f32, kind="ExternalOutput").ap()

    with tile.TileContext(nc) as tc, ExitStack() as ctx:
        xp = ctx.enter_context(tc.tile_pool(name="x", bufs=1))
        w13p = ctx.enter_context(tc.tile_pool(name="w13", bufs=3))
        hp = ctx.enter_context(tc.tile_pool(name="h", bufs=NI))
        silp = ctx.enter_context(tc.tile_pool(name="sil", bufs=2))
        w2p = ctx.enter_context(tc.tile_pool(name="w2", bufs=6))
        yp = ctx.enter_context(tc.tile_pool(name="y", bufs=1))

        # x in 8 per-k-slice DMAs so the first matmul only waits for slice 0
        xsb = xp.tile([128, ND * cap], DT)
        for k in range(ND):
            nc.sync.dma_start(
                xsb[:, k * cap : (k + 1) * cap], xt[:, k * cap : (k + 1) * cap]
            )

        hts = []
        with tc.tile_pool(name="hps", bufs=2, space="PSUM") as hps:
            for it in range(NI):
                wt = w13p.tile([128, 2 * D], DT)
                nc.sync.dma_start(wt[:], w13[it])
                h1 = hps.tile([128, cap], f32, tag="h1")
                h3 = hps.tile([128, cap], f32, tag="h3")
                for k in range(ND):
                    nc.tensor.matmul(
                        h1[:],
                        wt[:, k * 128 : (k + 1) * 128],
                        xsb[:, k * cap : (k + 1) * cap],
                        start=(k == 0),
                        stop=(k == ND - 1),
                    )
                for k in range(ND):
                    nc.tensor.matmul(
                        h3[:],
                        wt[:, D + k * 128 : D + (k + 1) * 128],
                        xsb[:, k * cap : (k + 1) * cap],
                        start=(k == 0),
                        stop=(k == ND - 1),
                    )
                sil = silp.tile([128, cap], f32)
                nc.scalar.activation(
                    sil[:], h1[:], mybir.ActivationFunctionType.Silu
                )
                ht = hp.tile([128, cap], DT)
                nc.vector.tensor_mul(ht[:], sil[:], h3[:])
                hts.append(ht)

        # Phase B: yT[d-tile][128, cap] += w2tile.T @ hT  (w2 stationary)
        with tc.tile_pool(name="yps", bufs=1, space="PSUM") as yps:
            ypt = [
                yps.tile([128, cap], f32, tag=f"yps_{k}", name=f"yps_{k}")
                for k in range(ND)
            ]
            for it in range(NI):
                w2t = w2p.tile([128, D], DT)
                nc.sync.dma_start(w2t[:], w2s[it])
                for k in range(ND):
                    nc.tensor.matmul(
                        ypt[k][:],
                        w2t[:, k * 128 : (k + 1) * 128],
                        hts[it][:],
                        start=(it == 0),
                        stop=(it == NI - 1),
                    )
            ysb = yp.tile([128, ND * cap], f32)
            for k in range(ND):
                nc.vector.tensor_copy(ysb[:, k * cap : (k + 1) * cap], ypt[k][:])
                nc.sync.dma_start(
                    yt[:, k * cap : (k + 1) * cap], ysb[:, k * cap : (k + 1) * cap]
                )

    nc.compile()
    return nc


def _get_program(cap: int, dt_name: str):
    key = (cap, dt_name)
    if key not in _PROG_CACHE:
        _PROG_CACHE[key] = _build_program(cap, dt_name)
    return _PROG_CACHE[key]


def _np_dt(dt_name: str):
    if dt_name == "float32":
        return np.float32
    import ml_dtypes

    return ml_dtypes.bfloat16


def _prep_weights(w1, w3, w2, dt_name):
    """Per-expert pretransposed/tiled weight arrays (see module docstring)."""
    npdt = _np_dt(dt_name)
    w13_all, w2s_all = [], []
    for e in range(E):
        # [I, D] -> [it, c, k, p] -> [it, p, k, c] -> [it, 128, 1024]
        a1 = w1[e].reshape(NI, 128, ND, 128).transpose(0, 3, 2, 1).reshape(NI, 128, D)
        a3 = w3[e].reshape(NI, 128, ND, 128).transpose(0, 3, 2, 1).reshape(NI, 128, D)
        w13_all.append(
            np.ascontiguousarray(np.concatenate([a1, a3], axis=2)).astype(npdt)
        )
        # w2[e] [D, I] -> T [I, D] -> [22, 128, 1024]
        w2s_all.append(
            np.ascontiguousarray(w2[e].T).reshape(NI, 128, D).astype(npdt)
        )
    return w13_all, w2s_all


def kernel(x, w1, w2, w3, expert_indices, _trace=False):
    x = np.asarray(x, dtype=np.float32)
    w1 = np.asarray(w1, dtype=np.float32)
    w2 = np.asarray(w2, dtype=np.float32)
    w3 = np.asarray(w3, dtype=np.float32)
    idx = np.asarray(expert_indices).astype(np.int64)
    T, A = idx.shape
    npdt = _np_dt(DT_NAME)

    flat = idx.ravel()  # position p = t*A + a -> expert id
    order = np.argsort(flat, kind="stable")
    counts = np.bincount(flat, minlength=E)
    offs = np.zeros(E + 1, dtype=np.int64)
    np.cumsum(counts, out=offs[1:])

    w13_all, w2s_all = _prep_weights(w1, w3, w2, DT_NAME)

    out = np.empty((T * A, D), dtype=np.float32)
    remaining = counts.copy()
    done = np.zeros(E, dtype=np.int64)
    last_res = None
    while remaining.max() > 0:
        cap = min(512, max(32, int(-(-remaining.max() // 16)) * 16))
        nc = _get_program(cap, DT_NAME)
        in_maps = []
        core_pos = []  # per-core flat positions handled this round
        for e in range(E):
            n = int(min(remaining[e], cap))
            pos = order[offs[e] + done[e] : offs[e] + done[e] + n]
            core_pos.append(pos)
            xg = np.zeros((cap, D), dtype=np.float32)
            xg[:n] = x[pos // A]
            # [cap, D] -> T [D, cap] -> [k, 128, cap] -> [128, k, cap]
            xt_host = np.ascontiguousarray(
                xg.T.reshape(ND, 128, cap).transpose(1, 0, 2)
            ).reshape(128, ND * cap).astype(npdt)
            in_maps.append({"xt": xt_host, "w13": w13_all[e], "w2s": w2s_all[e]})
            remaining[e] -= n
            done[e] += n
        last_res = run_bass_kernel_spmd(
            nc, in_maps, core_ids=list(range(N_CORES)), trace=_trace
        )
        for e in range(E):
            pos = core_pos[e]
            if len(pos):
                # yt [128, 8*cap] -> [p, k, j] -> y[j, k*128+p]
                ye = (
                    last_res.results[e]["yt"]
                    .reshape(128, ND, cap)
                    .transpose(2, 1, 0)
                    .reshape(cap, D)
                )
                out[pos] = ye[: len(pos)]

    result = out.reshape(T, A, D)
    if _trace:
        return result, last_res
    return result


# revision 7
# speedup vs baseline: 1.3790x; 1.0814x over previous
"""MoE ConditionalFeedForward (SwiGLU, top-2 of 8 experts) on 8 TRN2 NeuronCores.

Strategy: expert-parallel. Core e owns expert e's weights (w1/w2/w3 slices).
The host routes tokens: for each expert, gather the tokens assigned to it
(padded to CAP), each core computes
    y = (silu(x @ w1[e].T) * (x @ w3[e].T)) @ w2[e].T
densely for its gathered tokens, and the host scatters rows back into the
[T, A, D] output.

Per-core kernel layout (all host-pretransposed so every DMA is contiguous):
  xt  [128, 8*CAP]    xt[p, k*CAP+j] = x_g[j, k*128+p]     (tokens, transposed)
  w13 [22, 128, 2048] w13[it,p,k*128+c] = w1[e][it*128+c, k*128+p], w3 at +1024
  w2s [22, 128, 1024] w2s[it,p,d] = w2[e][d, it*128+p]
  yt  [128, 8*CAP]    f32, yt[p, k*CAP+j] = y_g[j, k*128+p] (output, transposed)

Phase A (per i-tile it of 22): h1T/h3T [128(i), CAP] = sum_k wT @ x tiles in
PSUM, then hT = silu(h1)*h3 into SBUF. Phase B (transposed): yT[d-tile] [128,
CAP] accumulated over the 22 i-tiles in PSUM (8 banks, one per d-tile), with
the w2 128x128 tile stationary and hT moving.
"""

import numpy as np
from contextlib import ExitStack

import concourse.bass as bass
import concourse.bacc as bacc
import concourse.mybir as mybir
import concourse.tile as tile
from concourse.bass_utils import run_bass_kernel_spmd

E, I, D = 8, 2816, 1024
N_CORES = 8
NI, ND = I // 128, D // 128  # 22, 8

# storage dtype for weights/activations on-device: "bfloat16" (half HBM
# traffic, full PE rate) or "float32" (matmuls run as float32r, 2 cyc/row)
DT_NAME = "bfloat16"

_PROG_CACHE: dict = {}


def _build_program(cap: int, dt_name: str):
    DT = mybir.dt.float32r if dt_name == "float32" else getattr(mybir.dt, dt_name)
    f32 = mybir.dt.float32
    NP = NI // 2  # w13/w2 DMAs batched as i-tile pairs for >=1MB transfers
    nc = bacc.Bacc("TRN2", target_bir_lowering=False, debug=False)
    xt = nc.dram_tensor("xt", [128, ND * cap], DT, kind="ExternalInput").ap()
    w13 = nc.dram_tensor("w13", [NP, 128, 4 * D], DT, kind="ExternalInput").ap()
    w2s = nc.dram_tensor("w2s", [NP, 128, 2 * D], DT, kind="ExternalInput").ap()
    yt = nc.dram_tensor("yt", [128, ND * cap], f32, kind="ExternalOutput").ap()
    warm_out = nc.dram_tensor("warm_out", [128, 512], f32, kind="ExternalOutput").ap()

    with tile.TileContext(nc) as tc, ExitStack() as ctx:
        warmp = ctx.enter_context(tc.tile_pool(name="warm", bufs=1))
        xp = ctx.enter_context(tc.tile_pool(name="x", bufs=1))
        w13p = ctx.enter_context(tc.tile_pool(name="w13", bufs=3))
        hp = ctx.enter_context(tc.tile_pool(name="h", bufs=NI))
        silp = ctx.enter_context(tc.tile_pool(name="sil", bufs=2))
        w2p = ctx.enter_context(tc.tile_pool(name="w2", bufs=1))
        yp = ctx.enter_context(tc.tile_pool(name="y", bufs=1))

        # PE warmup: ~12 matmuls on a zeroed tile, no DMA dependency, so the
        # HAM clock-gate is released during the initial weight-DMA window and
        # the real matmuls start at 2.4GHz.
        with tc.tile_pool(name="warmps", bufs=1, space="PSUM") as warmps:
            wtile = warmp.tile([128, 640], DT)
            nc.gpsimd.memset(wtile[:], 0.0)
            wps = warmps.tile([128, 512], f32)
            n_warm = 12
            for i in range(n_warm):
                nc.tensor.matmul(
                    wps[:],
                    wtile[:, 0:128],
                    wtile[:, 128:640],
                    start=(i == 0),
                    stop=(i == n_warm - 1),
                )
            wsc = warmp.tile([128, 512], f32)
            nc.vector.tensor_copy(wsc[:], wps[:])
            nc.gpsimd.dma_start(warm_out[:], wsc[:])

        # x: slice k=0 alone so the first matmul waits for only 72KB, rest in
        # one big transfer
        xsb = xp.tile([128, ND * cap], DT)
        nc.sync.dma_start(xsb[:, 0:cap], xt[:, 0:cap])
        nc.sync.dma_start(xsb[:, cap:], xt[:, cap:])

        # all w2 pairs up-front on the second HWDGE ring (Scalar) so phase B
        # never waits on DMA and the two rings stream in parallel
        w2ts = []
        for j in range(NP):
            w2t = w2p.tile([128, 2 * D], DT, tag=f"w2_{j}", name=f"w2_{j}")
            nc.scalar.dma_start(w2t[:], w2s[j])
            w2ts.append(w2t)

        hts = []
        with tc.tile_pool(name="hps", bufs=2, space="PSUM") as hps:
            for j in range(NP):
                wt = w13p.tile([128, 4 * D], DT)
                nc.sync.dma_start(wt[:], w13[j])
                for half in range(2):
                    base = half * 2 * D
                    h1 = hps.tile([128, cap], f32, tag="h1", name="h1")
                    h3 = hps.tile([128, cap], f32, tag="h3", name="h3")
                    for k in range(ND):
                        nc.tensor.matmul(
                            h1[:],
                            wt[:, base + k * 128 : base + (k + 1) * 128],
                            xsb[:, k * cap : (k + 1) * cap],
                            start=(k == 0),
                            stop=(k == ND - 1),
                        )
                    for k in range(ND):
                        nc.tensor.matmul(
                            h3[:],
                            wt[:, base + D + k * 128 : base + D + (k + 1) * 128],
                            xsb[:, k * cap : (k + 1) * cap],
                            start=(k == 0),
                            stop=(k == ND - 1),
                        )
                    sil = silp.tile([128, cap], f32)
                    nc.scalar.activation(
                        sil[:], h1[:], mybir.ActivationFunctionType.Silu
                    )
                    ht = hp.tile([128, cap], DT)
                    nc.vector.tensor_mul(ht[:], sil[:], h3[:])
                    hts.append(ht)

        # Phase B: yT[d-tile][128, cap] += w2tile.T @ hT  (w2 stationary)
        with tc.tile_pool(name="yps", bufs=1, space="PSUM") as yps:
            ypt = [
                yps.tile([128, cap], f32, tag=f"yps_{k}", name=f"yps_{k}")
                for k in range(ND)
            ]
            for it in range(NI):
                w2t = w2ts[it // 2]
                base = (it % 2) * D
                for k in range(ND):
                    nc.tensor.matmul(
                        ypt[k][:],
                        w2t[:, base + k * 128 : base + (k + 1) * 128],
                        hts[it][:],
                        start=(it == 0),
                        stop=(it == NI - 1),
                    )
            ysb = yp.tile([128, ND * cap], f32)
            for k in range(ND):
                dst = ysb[:, k * cap : (k + 1) * cap]
                if k % 2 == 0:
                    nc.vector.tensor_copy(dst, ypt[k][:])
                else:
                    nc.scalar.activation(
                        dst, ypt[k][:], mybir.ActivationFunctionType.Copy
                    )
                nc.sync.dma_start(yt[:, k * cap : (k + 1) * cap], dst)

    nc.compile()
    return nc


def _get_program(cap: int, dt_name: str):
    key = (cap, dt_name)
    if key not in _PROG_CACHE:
        _PROG_CACHE[key] = _build_program(cap, dt_name)
    return _PROG_CACHE[key]


def _np_dt(dt_name: str):
    if dt_name == "float32":
        return np.float32
    import ml_dtypes

    return ml_dtypes.bfloat16


def _prep_weights(w1, w3, w2, dt_name):
    """Per-expert pretransposed/tiled weight arrays (see module docstring)."""
    npdt = _np_dt(dt_name)
    w13_all, w2s_all = [], []
    for e in range(E):
        # [I, D] -> [it, c, k, p] -> [it, p, k, c] -> [it, 128, 1024]
        a1 = w1[e].reshape(NI, 128, ND, 128).transpose(0, 3, 2, 1).reshape(NI, 128, D)
        a3 = w3[e].reshape(NI, 128, ND, 128).transpose(0, 3, 2, 1).reshape(NI, 128, D)
        # pairs of i-tiles: [11, 128, 4096] = [w1|w3] for it=2j then it=2j+1
        a13 = np.concatenate([a1, a3], axis=2).reshape(NI // 2, 2, 128, 2 * D)
        w13_all.append(
            np.ascontiguousarray(a13.transpose(0, 2, 1, 3)).reshape(
                NI // 2, 128, 4 * D
            ).astype(npdt)
        )
        # w2[e] [D, I] -> T [I, D] -> [22, 128, 1024] -> pairs [11, 128, 2048]
        a2 = w2[e].T.reshape(NI // 2, 2, 128, D)
        w2s_all.append(
            np.ascontiguousarray(a2.transpose(0, 2, 1, 3)).reshape(
                NI // 2, 128, 2 * D
            ).astype(npdt)
        )
    return w13_all, w2s_all


def kernel(x, w1, w2, w3, expert_indices, _trace=False):
    x = np.asarray(x, dtype=np.float32)
    w1 = np.asarray(w1, dtype=np.float32)
    w2 = np.asarray(w2, dtype=np.float32)
    w3 = np.asarray(w3, dtype=np.float32)
    idx = np.asarray(expert_indices).astype(np.int64)
    T, A = idx.shape
    npdt = _np_dt(DT_NAME)

    flat = idx.ravel()  # position p = t*A + a -> expert id
    order = np.argsort(flat, kind="stable")
    counts = np.bincount(flat, minlength=E)
    offs = np.zeros(E + 1, dtype=np.int64)
    np.cumsum(counts, out=offs[1:])

    w13_all, w2s_all = _prep_weights(w1, w3, w2, DT_NAME)

    out = np.empty((T * A, D), dtype=np.float32)
    remaining = counts.copy()
    done = np.zeros(E, dtype=np.int64)
    last_res = None
    while remaining.max() > 0:
        cap = min(512, max(32, int(-(-remaining.max() // 16)) * 16))
        nc = _get_program(cap, DT_NAME)
        in_maps = []
        core_pos = []  # per-core flat positions handled this round
        for e in range(E):
            n = int(min(remaining[e], cap))
            pos = order[offs[e] + done[e] : offs[e] + done[e] + n]
            core_pos.append(pos)
            xg = np.zeros((cap, D), dtype=np.float32)
            xg[:n] = x[pos // A]
            # [cap, D] -> T [D, cap] -> [k, 128, cap] -> [128, k, cap]
            xt_host = np.ascontiguousarray(
                xg.T.reshape(ND, 128, cap).transpose(1, 0, 2)
            ).reshape(128, ND * cap).astype(npdt)
            in_maps.append({"xt": xt_host, "w13": w13_all[e], "w2s": w2s_all[e]})
            remaining[e] -= n
            done[e] += n
        last_res = run_bass_kernel_spmd(
            nc, in_maps, core_ids=list(range(N_CORES)), trace=_trace
        )
        for e in range(E):
            pos = core_pos[e]
            if len(pos):
                # yt [128, 8*cap] -> [p, k, j] -> y[j, k*128+p]
                ye = (
                    last_res.results[e]["yt"]
                    .reshape(128, ND, cap)
                    .transpose(2, 1, 0)
                    .reshape(cap, D)
                )
                out[pos] = ye[: len(pos)]

    result = out.reshape(T, A, D)
    if _trace:
        return result, last_res
    return result


# revision 10
# speedup vs baseline: 1.4120x; 1.0240x over previous
"""MoE ConditionalFeedForward (SwiGLU, top-2 of 8 experts) on 8 TRN2 NeuronCores.

Strategy: expert-parallel. Core e owns expert e's weights (w1/w2/w3 slices).
The host routes tokens: for each expert, gather the tokens assigned to it
(padded to CAP), each core computes
    y = (silu(x @ w1[e].T) * (x @ w3[e].T)) @ w2[e].T
densely for its gathered tokens, and the host scatters rows back into the
[T, A, D] output.

Per-core kernel layout (all host-pretransposed so every DMA is contiguous):
  xt  [128, 8*CAP]    xt[p, k*CAP+j] = x_g[j, k*128+p]     (tokens, transposed)
  w13 [22, 128, 2048] w13[it,p,k*128+c] = w1[e][it*128+c, k*128+p], w3 at +1024
  w2s [22, 128, 1024] w2s[it,p,d] = w2[e][d, it*128+p]
  yt  [128, 8*CAP]    f32, yt[p, k*CAP+j] = y_g[j, k*128+p] (output, transposed)

Phase A (per i-tile it of 22): h1T/h3T [128(i), CAP] = sum_k wT @ x tiles in
PSUM, then hT = silu(h1)*h3 into SBUF. Phase B (transposed): yT[d-tile] [128,
CAP] accumulated over the 22 i-tiles in PSUM (8 banks, one per d-tile), with
the w2 128x128 tile stationary and hT moving.
"""

import numpy as np
from contextlib import ExitStack

import concourse.bass as bass
import concourse.bacc as bacc
import concourse.mybir as mybir
import concourse.tile as tile
from concourse.bass_utils import run_bass_kernel_spmd

E, I, D = 8, 2816, 1024
N_CORES = 8
NI, ND = I // 128, D // 128  # 22, 8

# storage dtype for weights/activations on-device: "bfloat16" (half HBM
# traffic, full PE rate) or "float32" (matmuls run as float32r, 2 cyc/row)
DT_NAME = "bfloat16"

_PROG_CACHE: dict = {}


def _build_program(cap: int, dt_name: str):
    DT = mybir.dt.float32r if dt_name == "float32" else getattr(mybir.dt, dt_name)
    f32 = mybir.dt.float32
    NP = NI // 2  # w13/w2 DMAs batched as i-tile pairs for >=1MB transfers
    nc = bacc.Bacc("TRN2", target_bir_lowering=False, debug=False)
    xt = nc.dram_tensor("xt", [128, ND * cap], DT, kind="ExternalInput").ap()
    w13 = nc.dram_tensor("w13", [NP, 128, 4 * D], DT, kind="ExternalInput").ap()
    w2s = nc.dram_tensor("w2s", [NP, 128, 2 * D], DT, kind="ExternalInput").ap()
    yt = nc.dram_tensor("yt", [128, ND * cap], f32, kind="ExternalOutput").ap()
    warm_out = nc.dram_tensor("warm_out", [128, 512], f32, kind="ExternalOutput").ap()

    with tile.TileContext(nc) as tc, ExitStack() as ctx:
        warmp = ctx.enter_context(tc.tile_pool(name="warm", bufs=1))
        xp = ctx.enter_context(tc.tile_pool(name="x", bufs=1))
        w13p = ctx.enter_context(tc.tile_pool(name="w13", bufs=3))
        hp = ctx.enter_context(tc.tile_pool(name="h", bufs=NI))
        silp = ctx.enter_context(tc.tile_pool(name="sil", bufs=2))
        w2p = ctx.enter_context(tc.tile_pool(name="w2", bufs=1))
        yp = ctx.enter_context(tc.tile_pool(name="y", bufs=1))

        # PE warmup: ~12 matmuls on a zeroed tile, no DMA dependency, so the
        # HAM clock-gate is released during the initial weight-DMA window and
        # the real matmuls start at 2.4GHz.
        with tc.tile_pool(name="warmps", bufs=1, space="PSUM") as warmps:
            wtile = warmp.tile([128, 640], DT)
            nc.gpsimd.memset(wtile[:], 0.0)
            wps = warmps.tile([128, 512], f32)
            n_warm = 8
            for i in range(n_warm):
                nc.tensor.matmul(
                    wps[:],
                    wtile[:, 0:128],
                    wtile[:, 128:640],
                    start=(i == 0),
                    stop=(i == n_warm - 1),
                )
            wsc = warmp.tile([128, 512], f32)
            nc.vector.tensor_copy(wsc[:], wps[:])
            nc.gpsimd.dma_start(warm_out[:], wsc[:])

        # x: slice k=0 alone on the Sync ring (72KB, so the first matmul has a
        # short wait), the rest in parallel on the Scalar ring
        xsb = xp.tile([128, ND * cap], DT)
        nc.sync.dma_start(xsb[:, 0:cap], xt[:, 0:cap])
        nc.scalar.dma_start(xsb[:, cap:], xt[:, cap:])

        # w2 pair tiles: loaded on the Scalar ring, paced one pair per phase-A
        # pair so they don't compete with the critical w13 stream early on
        w2ts = [
            w2p.tile([128, 2 * D], DT, tag=f"w2_{j}", name=f"w2_{j}")
            for j in range(NP)
        ]

        hts = []
        with tc.tile_pool(name="hps", bufs=2, space="PSUM") as hps:
            for j in range(NP):
                wt = w13p.tile([128, 4 * D], DT)
                nc.sync.dma_start(wt[:], w13[j])
                nc.scalar.dma_start(w2ts[j][:], w2s[j])
                for half in range(2):
                    base = half * 2 * D
                    h1 = hps.tile([128, cap], f32, tag="h1", name="h1")
                    h3 = hps.tile([128, cap], f32, tag="h3", name="h3")
                    for k in range(ND):
                        nc.tensor.matmul(
                            h1[:],
                            wt[:, base + k * 128 : base + (k + 1) * 128],
                            xsb[:, k * cap : (k + 1) * cap],
                            start=(k == 0),
                            stop=(k == ND - 1),
                        )
                    for k in range(ND):
                        nc.tensor.matmul(
                            h3[:],
                            wt[:, base + D + k * 128 : base + D + (k + 1) * 128],
                            xsb[:, k * cap : (k + 1) * cap],
                            start=(k == 0),
                            stop=(k == ND - 1),
                        )
                    sil = silp.tile([128, cap], f32)
                    nc.scalar.activation(
                        sil[:], h1[:], mybir.ActivationFunctionType.Silu
                    )
                    ht = hp.tile([128, cap], DT)
                    nc.vector.tensor_mul(ht[:], sil[:], h3[:])
                    hts.append(ht)

        # Phase B: yT[d-tile][128, cap] += w2tile.T @ hT  (w2 stationary)
        with tc.tile_pool(name="yps", bufs=1, space="PSUM") as yps:
            ypt = [
                yps.tile([128, cap], f32, tag=f"yps_{k}", name=f"yps_{k}")
                for k in range(ND)
            ]
            # k-major: each d-tile's accumulation chain finishes early, so its
            # PSUM drain (copy + DMA out) overlaps the remaining matmuls
            ysb = yp.tile([128, ND * cap], f32)
            for k in range(ND):
                for it in range(NI):
                    nc.tensor.matmul(
                        ypt[k][:],
                        w2ts[it // 2][:, (it % 2) * D + k * 128 : (it % 2) * D + (k + 1) * 128],
                        hts[it][:],
                        start=(it == 0),
                        stop=(it == NI - 1),
                    )
                dst = ysb[:, k * cap : (k + 1) * cap]
                nc.vector.tensor_copy(dst, ypt[k][:])
                nc.sync.dma_start(yt[:, k * cap : (k + 1) * cap], dst)

    nc.compile()
    return nc


def _get_program(cap: int, dt_name: str):
    key = (cap, dt_name)
    if key not in _PROG_CACHE:
        _PROG_CACHE[key] = _build_program(cap, dt_name)
    return _PROG_CACHE[key]


def _np_dt(dt_name: str):
    if dt_name == "float32":
        return np.float32
    import ml_dtypes

    return ml_dtypes.bfloat16


def _prep_weights(w1, w3, w2, dt_name):
    """Per-expert pretransposed/tiled weight arrays (see module docstring)."""
    npdt = _np_dt(dt_name)
    w13_all, w2s_all = [], []
    for e in range(E):
        # [I, D] -> [it, c, k, p] -> [it, p, k, c] -> [it, 128, 1024]
        a1 = w1[e].reshape(NI, 128, ND, 128).transpose(0, 3, 2, 1).reshape(NI, 128, D)
        a3 = w3[e].reshape(NI, 128, ND, 128).transpose(0, 3, 2, 1).reshape(NI, 128, D)
        # pairs of i-tiles: [11, 128, 4096] = [w1|w3] for it=2j then it=2j+1
        a13 = np.concatenate([a1, a3], axis=2).reshape(NI // 2, 2, 128, 2 * D)
        w13_all.append(
            np.ascontiguousarray(a13.transpose(0, 2, 1, 3)).reshape(
                NI // 2, 128, 4 * D
            ).astype(npdt)
        )
        # w2[e] [D, I] -> T [I, D] -> [22, 128, 1024] -> pairs [11, 128, 2048]
        a2 = w2[e].T.reshape(NI // 2, 2, 128, D)
        w2s_all.append(
            np.ascontiguousarray(a2.transpose(0, 2, 1, 3)).reshape(
                NI // 2, 128, 2 * D
            ).astype(npdt)
        )
    return w13_all, w2s_all


def kernel(x, w1, w2, w3, expert_indices, _trace=False):
    x = np.asarray(x, dtype=np.float32)
    w1 = np.asarray(w1, dtype=np.float32)
    w2 = np.asarray(w2, dtype=np.float32)
    w3 = np.asarray(w3, dtype=np.float32)
    idx = np.asarray(expert_indices).astype(np.int64)
    T, A = idx.shape
    npdt = _np_dt(DT_NAME)

    flat = idx.ravel()  # position p = t*A + a -> expert id
    order = np.argsort(flat, kind="stable")
    counts = np.bincount(flat, minlength=E)
    offs = np.zeros(E + 1, dtype=np.int64)
    np.cumsum(counts, out=offs[1:])

    w13_all, w2s_all = _prep_weights(w1, w3, w2, DT_NAME)

    out = np.empty((T * A, D), dtype=np.float32)
    remaining = counts.copy()
    done = np.zeros(E, dtype=np.int64)
    last_res = None
    while remaining.max() > 0:
        cap = min(512, max(32, int(-(-remaining.max() // 16)) * 16))
        nc = _get_program(cap, DT_NAME)
        in_maps = []
        core_pos = []  # per-core flat positions handled this round
        for e in range(E):
            n = int(min(remaining[e], cap))
            pos = order[offs[e] + done[e] : offs[e] + done[e] + n]
            core_pos.append(pos)
            xg = np.zeros((cap, D), dtype=np.float32)
            xg[:n] = x[pos // A]
            # [cap, D] -> T [D, cap] -> [k, 128, cap] -> [128, k, cap]
            xt_host = np.ascontiguousarray(
                xg.T.reshape(ND, 128, cap).transpose(1, 0, 2)
            ).reshape(128, ND * cap).astype(npdt)
            in_maps.append({"xt": xt_host, "w13": w13_all[e], "w2s": w2s_all[e]})
            remaining[e] -= n
            done[e] += n
        last_res = run_bass_kernel_spmd(
            nc, in_maps, core_ids=list(range(N_CORES)), trace=_trace
        )
        for e in range(E):
            pos = core_pos[e]
            if len(pos):
                # yt [128, 8*cap] -> [p, k, j] -> y[j, k*128+p]
                ye = (
                    last_res.results[e]["yt"]
                    .reshape(128, ND, cap)
                    .transpose(2, 1, 0)
                    .reshape(cap, D)
                )
                out[pos] = ye[: len(pos)]

    result = out.reshape(T, A, D)
    if _trace:
        return result, last_res
    return result


# revision 11
# speedup vs baseline: 1.4921x; 1.0567x over previous
"""MoE ConditionalFeedForward (SwiGLU, top-2 of 8 experts) on 8 TRN2 NeuronCores.

Strategy: expert-parallel. Core e owns expert e's weights (w1/w2/w3 slices).
The host routes tokens: for each expert, gather the tokens assigned to it
(padded to CAP), each core computes
    y = (silu(x @ w1[e].T) * (x @ w3[e].T)) @ w2[e].T
densely for its gathered tokens, and the host scatters rows back into the
[T, A, D] output.

Per-core kernel layout (all host-pretransposed so every DMA is contiguous):
  xt  [128, 8*CAP]    xt[p, k*CAP+j] = x_g[j, k*128+p]     (tokens, transposed)
  w13 [22, 128, 2048] w13[it,p,k*128+c] = w1[e][it*128+c, k*128+p], w3 at +1024
  w2s [22, 128, 1024] w2s[it,p,d] = w2[e][d, it*128+p]
  yt  [128, 8*CAP]    f32, yt[p, k*CAP+j] = y_g[j, k*128+p] (output, transposed)

Phase A (per i-tile it of 22): h1T/h3T [128(i), CAP] = sum_k wT @ x tiles in
PSUM, then hT = silu(h1)*h3 into SBUF. Phase B (transposed): yT[d-tile] [128,
CAP] accumulated over the 22 i-tiles in PSUM (8 banks, one per d-tile), with
the w2 128x128 tile stationary and hT moving.
"""

import numpy as np
from contextlib import ExitStack

import concourse.bass as bass
import concourse.bacc as bacc
import concourse.mybir as mybir
import concourse.tile as tile
from concourse.bass_utils import run_bass_kernel_spmd

E, I, D = 8, 2816, 1024
N_CORES = 8
NI, ND = I // 128, D // 128  # 22, 8

# storage dtype for weights/activations on-device: "bfloat16" (half HBM
# traffic, full PE rate) or "float32" (matmuls run as float32r, 2 cyc/row)
DT_NAME = "bfloat16"

_PROG_CACHE: dict = {}


def _build_program(cap: int, dt_name: str):
    DT = mybir.dt.float32r if dt_name == "float32" else getattr(mybir.dt, dt_name)
    f32 = mybir.dt.float32
    NP = NI // 2  # w13/w2 DMAs batched as i-tile pairs for >=1MB transfers
    nc = bacc.Bacc("TRN2", target_bir_lowering=False, debug=False)
    xt = nc.dram_tensor("xt", [128, ND * cap], DT, kind="ExternalInput").ap()
    w13 = nc.dram_tensor("w13", [NP, 128, 4 * D], DT, kind="ExternalInput").ap()
    w2s = nc.dram_tensor("w2s", [NP, 128, 2 * D], DT, kind="ExternalInput").ap()
    yt = nc.dram_tensor("yt", [128, ND * cap], f32, kind="ExternalOutput").ap()
    warm_out = nc.dram_tensor("warm_out", [128, 16], f32, kind="ExternalOutput").ap()

    with tile.TileContext(nc) as tc, ExitStack() as ctx:
        warmp = ctx.enter_context(tc.tile_pool(name="warm", bufs=1))
        xp = ctx.enter_context(tc.tile_pool(name="x", bufs=1))
        w13p = ctx.enter_context(tc.tile_pool(name="w13", bufs=3))
        hp = ctx.enter_context(tc.tile_pool(name="h", bufs=NI))
        silp = ctx.enter_context(tc.tile_pool(name="sil", bufs=2))
        w2p = ctx.enter_context(tc.tile_pool(name="w2", bufs=1))
        yp = ctx.enter_context(tc.tile_pool(name="y", bufs=1))

        # x on the Scalar HWDGE ring; w13 stream (critical path) alone on the
        # Sync ring. Emitted first for top scheduler priority.
        xsb = xp.tile([128, ND * cap], DT)
        nc.scalar.dma_start(xsb[:, 0:cap], xt[:, 0:cap])
        nc.scalar.dma_start(xsb[:, cap:], xt[:, cap:])
        w13ts = []
        for j in range(3):
            wt = w13p.tile([128, 4 * D], DT, tag="w13", name=f"w13_{j}")
            nc.sync.dma_start(wt[:], w13[j])
            w13ts.append(wt)

        # PE warmup: 8 matmuls on a zeroed tile, no DMA dependency, so the
        # HAM clock-gate is released during the initial weight-DMA window and
        # the real matmuls start at 2.4GHz.
        with tc.tile_pool(name="warmps", bufs=1, space="PSUM") as warmps:
            wtile = warmp.tile([128, 640], DT)
            nc.gpsimd.memset(wtile[:], 0.0)
            wps = warmps.tile([128, 512], f32)
            n_warm = 8
            for i in range(n_warm):
                nc.tensor.matmul(
                    wps[:],
                    wtile[:, 0:128],
                    wtile[:, 128:640],
                    start=(i == 0),
                    stop=(i == n_warm - 1),
                )
            wsc = warmp.tile([128, 16], f32)
            nc.vector.tensor_copy(wsc[:], wps[:, 0:16])
            nc.gpsimd.dma_start(warm_out[:], wsc[:])

        # w2 pair tiles on the Scalar ring: pairs 0-5 paced through phase A
        # (needed when phase B starts), pairs 6-10 paced through phase B's
        # first half, keeping phase A's DMA window under the HBM ceiling
        w2ts = [
            w2p.tile([128, 2 * D], DT, tag=f"w2_{j}", name=f"w2_{j}")
            for j in range(NP)
        ]

        hts = []
        with tc.tile_pool(name="hps", bufs=2, space="PSUM") as hps:
            for j in range(NP):
                if j < 3:
                    wt = w13ts[j]
                else:
                    wt = w13p.tile([128, 4 * D], DT, tag="w13", name=f"w13_{j}")
                    nc.sync.dma_start(wt[:], w13[j])
                if j < 6:
                    nc.scalar.dma_start(w2ts[j][:], w2s[j])
                for half in range(2):
                    base = half * 2 * D
                    h1 = hps.tile([128, cap], f32, tag="h1", name="h1")
                    h3 = hps.tile([128, cap], f32, tag="h3", name="h3")
                    for k in range(ND):
                        nc.tensor.matmul(
                            h1[:],
                            wt[:, base + k * 128 : base + (k + 1) * 128],
                            xsb[:, k * cap : (k + 1) * cap],
                            start=(k == 0),
                            stop=(k == ND - 1),
                        )
                    for k in range(ND):
                        nc.tensor.matmul(
                            h3[:],
                            wt[:, base + D + k * 128 : base + D + (k + 1) * 128],
                            xsb[:, k * cap : (k + 1) * cap],
                            start=(k == 0),
                            stop=(k == ND - 1),
                        )
                    sil = silp.tile([128, cap], f32)
                    nc.scalar.activation(
                        sil[:], h1[:], mybir.ActivationFunctionType.Silu
                    )
                    ht = hp.tile([128, cap], DT)
                    nc.vector.tensor_mul(ht[:], sil[:], h3[:])
                    hts.append(ht)

        # Phase B: yT[d-tile][128, cap] += w2tile.T @ hT (w2 stationary),
        # k-major within each it-half so each d-tile's PSUM drain overlaps the
        # remaining matmuls; two it-halves so w2 pairs 6-10 stream during the
        # first half
        NH = NI // 2  # 11
        with tc.tile_pool(name="yps", bufs=1, space="PSUM") as yps:
            ypt = [
                yps.tile([128, cap], f32, tag=f"yps_{k}", name=f"yps_{k}")
                for k in range(ND)
            ]
            ysb = yp.tile([128, ND * cap], f32)
            for k in range(ND):
                if 6 + k <= 10:
                    nc.scalar.dma_start(w2ts[6 + k][:], w2s[6 + k])
                for it in range(NH):
                    nc.tensor.matmul(
                        ypt[k][:],
                        w2ts[it // 2][:, (it % 2) * D + k * 128 : (it % 2) * D + (k + 1) * 128],
                        hts[it][:],
                        start=(it == 0),
                        stop=False,
                    )
            for k in range(ND):
                for it in range(NH, NI):
                    nc.tensor.matmul(
                        ypt[k][:],
                        w2ts[it // 2][:, (it % 2) * D + k * 128 : (it % 2) * D + (k + 1) * 128],
                        hts[it][:],
                        start=False,
                        stop=(it == NI - 1),
                    )
                dst = ysb[:, k * cap : (k + 1) * cap]
                nc.vector.tensor_copy(dst, ypt[k][:])
                nc.sync.dma_start(yt[:, k * cap : (k + 1) * cap], dst)

    nc.compile()
    return nc


def _get_program(cap: int, dt_name: str):
    key = (cap, dt_name)
    if key not in _PROG_CACHE:
        _PROG_CACHE[key] = _build_program(cap, dt_name)
    return _PROG_CACHE[key]


def _np_dt(dt_name: str):
    if dt_name == "float32":
        return np.float32
    import ml_dtypes

    return ml_dtypes.bfloat16


def _prep_weights(w1, w3, w2, dt_name):
    """Per-expert pretransposed/tiled weight arrays (see module docstring)."""
    npdt = _np_dt(dt_name)
    w13_all, w2s_all = [], []
    for e in range(E):
        # [I, D] -> [it, c, k, p] -> [it, p, k, c] -> [it, 128, 1024]
        a1 = w1[e].reshape(NI, 128, ND, 128).transpose(0, 3, 2, 1).reshape(NI, 128, D)
        a3 = w3[e].reshape(NI, 128, ND, 128).transpose(0, 3, 2, 1).reshape(NI, 128, D)
        # pairs of i-tiles: [11, 128, 4096] = [w1|w3] for it=2j then it=2j+1
        a13 = np.concatenate([a1, a3], axis=2).reshape(NI // 2, 2, 128, 2 * D)
        w13_all.append(
            np.ascontiguousarray(a13.transpose(0, 2, 1, 3)).reshape(
                NI // 2, 128, 4 * D
            ).astype(npdt)
        )
        # w2[e] [D, I] -> T [I, D] -> [22, 128, 1024] -> pairs [11, 128, 2048]
        a2 = w2[e].T.reshape(NI // 2, 2, 128, D)
        w2s_all.append(
            np.ascontiguousarray(a2.transpose(0, 2, 1, 3)).reshape(
                NI // 2, 128, 2 * D
            ).astype(npdt)
        )
    return w13_all, w2s_all


def kernel(x, w1, w2, w3, expert_indices, _trace=False):
    x = np.asarray(x, dtype=np.float32)
    w1 = np.asarray(w1, dtype=np.float32)
    w2 = np.asarray(w2, dtype=np.float32)
    w3 = np.asarray(w3, dtype=np.float32)
    idx = np.asarray(expert_indices).astype(np.int64)
    T, A = idx.shape
    npdt = _np_dt(DT_NAME)

    flat = idx.ravel()  # position p = t*A + a -> expert id
    order = np.argsort(flat, kind="stable")
    counts = np.bincount(flat, minlength=E)
    offs = np.zeros(E + 1, dtype=np.int64)
    np.cumsum(counts, out=offs[1:])

    w13_all, w2s_all = _prep_weights(w1, w3, w2, DT_NAME)

    out = np.empty((T * A, D), dtype=np.float32)
    remaining = counts.copy()
    done = np.zeros(E, dtype=np.int64)
    last_res = None
    while remaining.max() > 0:
        cap = min(512, max(32, int(-(-remaining.max() // 16)) * 16))
        nc = _get_program(cap, DT_NAME)
        in_maps = []
        core_pos = []  # per-core flat positions handled this round
        for e in range(E):
            n = int(min(remaining[e], cap))
            pos = order[offs[e] + done[e] : offs[e] + done[e] + n]
            core_pos.append(pos)
            xg = np.zeros((cap, D), dtype=np.float32)
            xg[:n] = x[pos // A]
            # [cap, D] -> T [D, cap] -> [k, 128, cap] -> [128, k, cap]
            xt_host = np.ascontiguousarray(
                xg.T.reshape(ND, 128, cap).transpose(1, 0, 2)
            ).reshape(128, ND * cap).astype(npdt)
            in_maps.append({"xt": xt_host, "w13": w13_all[e], "w2s": w2s_all[e]})
            remaining[e] -= n
            done[e] += n
        last_res = run_bass_kernel_spmd(
            nc, in_maps, core_ids=list(range(N_CORES)), trace=_trace
        )
        for e in range(E):
            pos = core_pos[e]
            if len(pos):
                # yt [128, 8*cap] -> [p, k, j] -> y[j, k*128+p]
                ye = (
                    last_res.results[e]["yt"]
                    .reshape(128, ND, cap)
                    .transpose(2, 1, 0)
                    .reshape(cap, D)
                )
                out[pos] = ye[: len(pos)]

    result = out.reshape(T, A, D)
    if _trace:
        return result, last_res
    return result
